# revision 1
# baseline (speedup 1.0000x reference)
"""Multi-head attention (B=4, S=2048, D=1024, H=16) on 8 trn2 NeuronCores.

Sharding: 8 cores = 4 batches x 2 head-groups. Core c handles batch c//2 and
heads [8g, 8g+8) where g = c%2 (tensor-parallel: Wq/Wk/Wv column-sliced,
Wo row-sliced). Each core returns a partial output [S, D]; the host sums the
two head-group partials per batch.

Per-core dataflow (everything stays transposed until the output projection):
  xT tiles (PE transpose) -> Q.T/K.T = W.T @ X.T (bf16), V natural (bf16,
  ones column appended) -> scores.T = K @ Q.T -> exp+mask+scale in one ACT op
  -> ctxU.T = V'.T @ expS.T (last row = softmax denominator) -> normalize ->
  out = ctx.T.T @ Wo (fp32r) + bo.
"""

import sys

if "/opt/trn_rl_repo" not in sys.path:
    sys.path.append("/opt/trn_rl_repo")

import numpy as np

import concourse.bass as bass
import concourse.bacc as bacc
import concourse.tile as tile
from concourse import mybir
from concourse.bass import ts
from concourse.masks import make_identity

F32 = mybir.dt.float32
F32R = mybir.dt.float32r
BF16 = mybir.dt.bfloat16
I32 = mybir.dt.int32
EXP = mybir.ActivationFunctionType.Exp

P = 128


def build_nc(S=2048, D=1024, DL=512, HD=64):
    """Build the per-core Bass program. DL = local output dim (heads*HD)."""
    ST = S // P  # token tiles
    KD = D // P  # contraction tiles over D
    MT = DL // P  # local d-col tiles
    HL = DL // HD  # local heads
    HPT = P // HD  # heads per 128-partition tile (2)
    NCH = min(512, S)  # projection token-chunk
    QS = min(1024, S)  # attention q superchunk (<=2 psum banks)
    QH = min(512, QS)  # one-bank half
    NH = QS // QH
    NQ = S // QS  # q-superchunks
    OC = min(512, D)  # out-proj col chunk
    scale = float(1.0 / (np.sqrt(np.float32(HD)) + 1e-8))

    nc = bacc.Bacc("TRN2", target_bir_lowering=False, debug=False)

    xq = nc.dram_tensor("xq", [S, D], F32, kind="ExternalInput")
    xk = nc.dram_tensor("xk", [S, D], F32, kind="ExternalInput")
    xv = nc.dram_tensor("xv", [S, D], F32, kind="ExternalInput")
    msk = nc.dram_tensor("msk", [P, ST], I32, kind="ExternalInput")
    wq = nc.dram_tensor("wq", [D, DL], F32, kind="ExternalInput")
    wk = nc.dram_tensor("wk", [D, DL], F32, kind="ExternalInput")
    wv = nc.dram_tensor("wv", [D, DL], F32, kind="ExternalInput")
    wo = nc.dram_tensor("wo", [DL, D], F32, kind="ExternalInput")
    bq = nc.dram_tensor("bq", [P, MT], F32, kind="ExternalInput")
    bk = nc.dram_tensor("bk", [P, MT], F32, kind="ExternalInput")
    bv = nc.dram_tensor("bv", [1, DL], F32, kind="ExternalInput")
    bo = nc.dram_tensor("bo", [1, D], F32, kind="ExternalInput")
    out = nc.dram_tensor("out", [S, D], F32, kind="ExternalOutput")

    with tile.TileContext(nc) as tc, nc.allow_low_precision("fp32r matmul operands are rounded by design"):
        with (
            tc.tile_pool(name="pers", bufs=1) as pers,
            tc.tile_pool(name="wpool", bufs=1) as wpool,
            tc.tile_pool(name="xnat", bufs=3) as xnat_pool,
            tc.tile_pool(name="xt", bufs=KD + 1) as xt_pool,
            tc.tile_pool(name="xtc", bufs=6) as xtc_pool,
            tc.tile_pool(name="exp", bufs=10) as ex_pool,
            tc.tile_pool(name="osb", bufs=2) as osb_pool,
            tc.tile_pool(name="small", bufs=2) as small,
        ):
            # ---- constants ----
            ident = pers.tile([P, P], F32, tag="ident")
            make_identity(nc, ident[:])
            ones0 = pers.tile([1, P], F32, tag="ones0")
            nc.gpsimd.memset(ones0[:], 1.0)
            ones = pers.tile([1, P], F32R, tag="ones")
            nc.vector.tensor_copy(out=ones[:], in_=ones0[:])

            mi = pers.tile([P, ST], I32, tag="mi")
            nc.sync.dma_start(mi[:], msk[:, :])
            mf = pers.tile([P, ST], F32, tag="mf")
            nc.vector.tensor_copy(out=mf[:], in_=mi[:])
            mb = pers.tile([P, ST], F32, tag="mb")
            nc.vector.tensor_scalar_mul(mb[:], mf[:], -1.0e9)

            bqs = pers.tile([P, MT], F32, tag="bqs")
            nc.sync.dma_start(bqs[:], bq[:, :])
            bks = pers.tile([P, MT], F32, tag="bks")
            nc.sync.dma_start(bks[:], bk[:, :])
            bvstg = small.tile([1, D], F32, tag="bstg", name="bvstg")
            nc.sync.dma_start(bvstg[0:1, 0:DL], bv[:, :])
            bvs = pers.tile([1, DL], F32R, tag="bvs")
            nc.vector.tensor_copy(out=bvs[:], in_=bvstg[0:1, 0:DL])
            bostg = small.tile([1, D], F32, tag="bstg", name="bostg")
            nc.sync.dma_start(bostg[:], bo[:, :])
            bos = pers.tile([1, D], F32R, tag="bos")
            nc.vector.tensor_copy(out=bos[:], in_=bostg[:])
            bvb = pers.tile([P, DL], F32, tag="bvb")
            bob = pers.tile([P, D], F32, tag="bob")

            wstg0 = wpool.tile([P, MT, D], F32, tag="wstg", name="wstg0")
            nc.sync.dma_start(wstg0[:], wo.rearrange("(m p) n -> p m n", p=P))
            wos = pers.tile([P, MT, D], BF16, tag="wos")
            nc.vector.tensor_copy(out=wos[:], in_=wstg0[:])

            # persistent activation stores
            KT = [pers.tile([P, S], BF16, tag=f"kt{m}", name=f"kt{m}") for m in range(MT)]
            QT = [pers.tile([P, S], BF16, tag=f"qt{m}", name=f"qt{m}") for m in range(MT)]
            CT = [pers.tile([P, S], BF16, tag=f"ct{m}", name=f"ct{m}") for m in range(MT)]
            VP = [pers.tile([P, HL * (HD + 1)], BF16, tag=f"vp{t}", name=f"vp{t}") for t in range(ST)]
            for t in range(ST):
                nc.gpsimd.memset(VP[t][:], 1.0)

            def load_w(wdram):
                stg = wpool.tile([P, KD, DL], F32, tag="wstg", name="wstg")
                nc.sync.dma_start(stg[:], wdram.rearrange("(k p) n -> p k n", p=P))
                w = wpool.tile([P, KD, DL], F32R, tag="w", name="w")
                nc.vector.tensor_copy(out=w[:], in_=stg[:])
                return w

            def tpose(dst, src, tp_slot):
                """dst[128, 128] (SBUF) = src[128, 128].T via PE."""
                nc.tensor.transpose(tp_slot, src, ident[:])
                nc.vector.tensor_copy(out=dst, in_=tp_slot)

            def proj_units(xdram, wsb, bias_sb, dst_tiles, nch, tp_pool, acc_pool):
                """dst[m][:, nch-chunk] = (x @ w + b).T; yields at unit edges."""
                nt = NCH // P
                xts = [
                    xt_pool.tile([P, NCH], F32R, tag="xt", name="xt") for _ in range(KD)
                ]
                for i in range(nt):
                    xn = xnat_pool.tile([P, D], F32, tag="xnat")
                    nc.sync.dma_start(xn[:], xdram[ts(nch * nt + i, P), :])
                    tp4 = (tp_pool if i % 2 == 0 else acc_pool).tile(
                        [P, 4, P], F32, tag="tp" if i % 2 == 0 else "acc", name="tp4"
                    )
                    for kk in range(KD):
                        tpose(xts[kk][:, ts(i, P)], xn[:, ts(kk, P)], tp4[:, kk % 4, :])
                    yield
                for m in range(MT):
                    acc = acc_pool.tile([P, NCH], F32, tag="acc")
                    for kk in range(KD):
                        nc.tensor.matmul(
                            acc[:],
                            lhsT=wsb[:, kk, ts(m, P)],
                            rhs=xts[kk][:],
                            start=(kk == 0),
                            stop=(kk == KD - 1),
                        )
                    nc.vector.tensor_scalar_add(
                        dst_tiles[m][:, ts(nch, NCH)], acc[:], bias_sb[:, m : m + 1]
                    )
                    yield

            def proj_T(xdram, wsb, bias_sb, dst_tiles, nch, tp_pool, acc_pool):
                for _ in proj_units(
                    xdram, wsb, bias_sb, dst_tiles, nch, tp_pool, acc_pool
                ):
                    pass

            def vproj(wsb, tp_pool, acc_pool):
                """VP[t][:, h*(HD+1):+HD] = (xv @ wv + bv)[t-tile, h-slice]."""
                for t in range(ST):
                    xn = xnat_pool.tile([P, D], F32, tag="xnat")
                    nc.sync.dma_start(xn[:], xv[ts(t, P), :])
                    xts = []
                    tp4 = tp_pool.tile([P, 4, P], F32, tag="tp", name="tp4")
                    for kk in range(KD):
                        xt = xtc_pool.tile([P, P], F32R, tag="xtc")
                        tpose(xt[:], xn[:, ts(kk, P)], tp4[:, kk % 4, :])
                        xts.append(xt)
                    acc = acc_pool.tile([P, DL], F32, tag="acc")
                    for kk in range(KD):
                        nc.tensor.matmul(
                            acc[:],
                            lhsT=xts[kk][:],
                            rhs=wsb[:, kk, :],
                            start=(kk == 0),
                            stop=(kk == KD - 1),
                        )
                    for h in range(HL):
                        nc.vector.tensor_add(
                            VP[t][:, h * (HD + 1) : h * (HD + 1) + HD],
                            acc[:, ts(h, HD)],
                            bvb[:, ts(h, HD)],
                        )

            def attention(qq, sc_pool, cx_pool, tp_pool, filler=None, pump_every=8):
                it = 0
                pending = []  # deferred normalize work (recip/broadcast/mul)
                for hp in range(HL // HPT):  # head pairs share a KT/QT tile
                    for q5 in range(NH):
                        col0 = qq * QS + q5 * QH
                        cxs = [
                            cx_pool.tile([HD + 1, QH], F32, tag="cx", name="cx")
                            for _ in range(HPT)
                        ]
                        for kt in range(ST):
                            # one PSUM supertile holds both heads' score chunk;
                            # the two K=64 matmuls run concurrently (row groups
                            # 0-63 / 64-127), one ACT exp covers both
                            sc = sc_pool.tile([P, HPT * QH], F32, tag="sc")
                            for u in range(HPT):
                                mo = u * HD
                                nc.tensor.matmul(
                                    sc[:, ts(u, QH)],
                                    lhsT=KT[hp][mo : mo + HD, ts(kt, P)],
                                    rhs=QT[hp][mo : mo + HD, col0 : col0 + QH],
                                    start=True,
                                    stop=True,
                                )
                            ex = ex_pool.tile([P, HPT * QH], BF16, tag="ex")
                            nc.scalar.activation(
                                ex[:], sc[:], EXP, bias=mb[:, kt : kt + 1], scale=scale
                            )
                            for u in range(HPT):
                                h = hp * HPT + u
                                nc.tensor.matmul(
                                    cxs[u][:],
                                    lhsT=VP[kt][:, h * (HD + 1) : (h + 1) * (HD + 1)],
                                    rhs=ex[:, ts(u, QH)],
                                    start=(kt == 0),
                                    stop=(kt == ST - 1),
                                )
                            it += 1
                            if filler is not None and it % pump_every == 0:
                                next(filler, None)
                        prev_tails = pending
                        pending = []
                        for u in range(HPT):
                            mo = u * HD
                            # the cheap DVE copy (emitted now, ahead of the
                            # previous unit's reciprocals in DVE order) frees
                            # the ctx PSUM slot; recip/broadcast/mul are
                            # deferred one unit so nothing waits on them
                            stg = small.tile([HD + 1, QH], F32, tag="stg", name="stg", bufs=4)
                            nc.vector.tensor_copy(out=stg[:], in_=cxs[u][:])

                            def tail(hp=hp, mo=mo, col0=col0, stg=stg):
                                rec = small.tile([1, QH], F32, tag="rec", name="rec", bufs=2)
                                nc.vector.reciprocal(rec[:], stg[HD : HD + 1, :])
                                bcs = small.tile([HD, QH], F32, tag="bcs", bufs=2)
                                nc.gpsimd.partition_broadcast(bcs[:], rec[0:1, :])
                                if mo == 0:
                                    nc.vector.tensor_mul(
                                        CT[hp][0:HD, col0 : col0 + QH],
                                        stg[0:HD, :],
                                        bcs[:],
                                    )
                                else:
                                    tmp = small.tile([HD, QH], BF16, tag="tmp")
                                    nc.vector.tensor_mul(tmp[:], stg[0:HD, :], bcs[:])
                                    nc.sync.dma_start(
                                        CT[hp][mo : mo + HD, col0 : col0 + QH], tmp[:]
                                    )

                            pending.append(tail)
                        for fn in prev_tails:
                            fn()

                for fn in pending:
                    fn()

            def outproj_units(qq, tp_pool, acc_pool):
                t0 = qq * (QS // P)
                for t in range(t0, t0 + QS // P):
                    for c in range(D // OC):
                        even = (t * (D // OC) + c) % 2 == 0
                        po = (tp_pool if even else acc_pool).tile(
                            [P, OC], F32, tag="tp" if even else "acc", name="po"
                        )
                        for dd in range(MT):
                            nc.tensor.matmul(
                                po[:],
                                lhsT=CT[dd][:, ts(t, P)],
                                rhs=wos[:, dd, ts(c, OC)],
                                start=(dd == 0),
                                stop=(dd == MT - 1),
                            )
                        osb = osb_pool.tile([P, OC], F32, tag="osb")
                        nc.vector.tensor_add(osb[:], po[:], bob[:, ts(c, OC)])
                        nc.sync.dma_start(out[ts(t, P), ts(c, OC)], osb[:])
                        yield

            def outproj(qq, tp_pool, acc_pool):
                for _ in outproj_units(qq, tp_pool, acc_pool):
                    pass

            # ---- phase 1: K.T and V' (full-S prerequisites of attention) ----
            with (
                tc.tile_pool(name="ps1tp", bufs=3, space="PSUM") as ps1tp,
                tc.tile_pool(name="ps1acc", bufs=4, space="PSUM") as ps1acc,
            ):
                for c in range(D // OC):
                    bp = ps1acc.tile([P, OC], F32, tag="acc", name="bp")
                    nc.tensor.matmul(
                        bp[:], lhsT=ones[0:1, 0:P], rhs=bos[0:1, ts(c, OC)],
                        start=True, stop=True,
                    )
                    nc.vector.tensor_copy(out=bob[:, ts(c, OC)], in_=bp[:])
                for c in range(DL // min(OC, DL)):
                    w_ = min(OC, DL)
                    bp = ps1acc.tile([P, w_], F32, tag="acc", name="bp2")
                    nc.tensor.matmul(
                        bp[:], lhsT=ones[0:1, 0:P], rhs=bvs[0:1, ts(c, w_)],
                        start=True, stop=True,
                    )
                    nc.vector.tensor_copy(out=bvb[:, ts(c, w_)], in_=bp[:])
                wks = load_w(wk)
                for nch in range(S // NCH):
                    proj_T(xk, wks, bks, KT, nch, ps1tp, ps1acc)
                wvs = load_w(wv)
                vproj(wvs, ps1tp, ps1acc)

            # ---- phase 2: Q.T chunks, attention, out-proj ----
            with (
                tc.tile_pool(name="ps2tp", bufs=1, space="PSUM") as ps2tp,
                tc.tile_pool(name="ps2acc", bufs=1, space="PSUM") as ps2acc,
                tc.tile_pool(name="ps2sc", bufs=2, space="PSUM") as ps2sc,
                tc.tile_pool(name="ps2cx", bufs=2, space="PSUM") as ps2cx,
            ):
                wqs = load_w(wq)
                CPQ = QS // NCH  # projection chunks per q-superchunk
                from itertools import chain

                for nch in range(CPQ):
                    proj_T(xq, wqs, bqs, QT, nch, ps2tp, ps2acc)
                for qq in range(NQ):
                    if qq + 1 < NQ:
                        filler = chain.from_iterable(
                            proj_units(xq, wqs, bqs, QT, nch, ps2tp, ps2acc)
                            for nch in range((qq + 1) * CPQ, (qq + 2) * CPQ)
                        )
                        n_units = CPQ * (NCH // P + MT)
                    elif qq >= 1:
                        filler = outproj_units(qq - 1, ps2tp, ps2acc)
                        n_units = (QS // P) * (D // OC)
                    else:
                        filler = None
                        n_units = 1
                    attention(
                        qq,
                        ps2sc,
                        ps2cx,
                        ps2tp,
                        filler,
                        pump_every=max(1, (HL * ST) // max(n_units, 1)),
                    )
                    if filler is not None:
                        for _ in filler:
                            pass
                outproj(NQ - 1, ps2tp, ps2acc)
                if NQ == 1:
                    pass
                else:
                    for qq in range(NQ - 2):
                        outproj(qq, ps2tp, ps2acc)

    nc.compile()
    return nc


_NC_CACHE = {}


def _get_nc(S, D, DL, HD):
    key = (S, D, DL, HD)
    if key not in _NC_CACHE:
        _NC_CACHE[key] = build_nc(S, D, DL, HD)
    return _NC_CACHE[key]


def _shard_inputs(q, k, v, mask, Wq, bq, Wk, bk, Wv, bv, Wo, bo):
    q, k, v = np.asarray(q), np.asarray(k), np.asarray(v)
    mask = np.asarray(mask)
    Wq, Wk, Wv, Wo = np.asarray(Wq), np.asarray(Wk), np.asarray(Wv), np.asarray(Wo)
    bq, bk, bv, bo = np.asarray(bq), np.asarray(bk), np.asarray(bv), np.asarray(bo)

    B, S, D = q.shape  # 4, 2048, 1024
    G = 2  # head-groups (tensor-parallel factor); B*G = 8 cores
    DL = D // G
    MT = DL // P
    ST = S // P

    f32 = np.float32
    in_maps = []
    for c in range(B * G):
        b, g = c // G, c % G
        sl = slice(g * DL, (g + 1) * DL)
        bo_core = bo if g == 0 else np.zeros_like(bo)
        in_maps.append(
            {
                "xq": np.ascontiguousarray(q[b], dtype=f32),
                "xk": np.ascontiguousarray(k[b], dtype=f32),
                "xv": np.ascontiguousarray(v[b], dtype=f32),
                "msk": np.ascontiguousarray(
                    mask[b, 0, 0].reshape(ST, P).T, dtype=np.int32
                ),
                "wq": np.ascontiguousarray(Wq[:, sl], dtype=f32),
                "wk": np.ascontiguousarray(Wk[:, sl], dtype=f32),
                "wv": np.ascontiguousarray(Wv[:, sl], dtype=f32),
                "wo": np.ascontiguousarray(Wo[sl, :], dtype=f32),
                "bq": np.ascontiguousarray(bq[sl].reshape(MT, P).T, dtype=f32),
                "bk": np.ascontiguousarray(bk[sl].reshape(MT, P).T, dtype=f32),
                "bv": np.ascontiguousarray(bv[sl].reshape(1, DL), dtype=f32),
                "bo": np.ascontiguousarray(bo_core.reshape(1, D), dtype=f32),
            }
        )
    return in_maps


def kernel(q, k, v, mask, Wq, bq, Wk, bk, Wv, bv, Wo, bo):
    from concourse.bass_utils import run_bass_kernel_spmd

    q = np.asarray(q)
    B, S, D = q.shape  # 4, 2048, 1024
    G = 2
    nc = _get_nc(S, D, D // G, 64)
    in_maps = _shard_inputs(q, k, v, mask, Wq, bq, Wk, bk, Wv, bv, Wo, bo)

    res = run_bass_kernel_spmd(nc, in_maps, core_ids=list(range(B * G)))
    parts = [r["out"] for r in res.results]
    outf = np.stack([parts[b * G] + parts[b * G + 1] for b in range(B)], axis=0)
    return outf.astype(np.float32)



# revision 4
# speedup vs baseline: 1.8601x; 1.8601x over previous
"""Multi-head attention (B=4, S=2048, D=1024, H=16) on 8 trn2 NeuronCores.

Sharding: 8 cores = 4 batches x 2 head-groups. Core c handles batch c//2 and
heads [8g, 8g+8) where g = c%2 (tensor-parallel: Wq/Wk/Wv column-sliced,
Wo row-sliced). Each core returns a partial output [S, D]; the host sums the
two head-group partials per batch.

Host-side prep: keys/values are COMPACTED per batch (mask==1 keys contribute
exp(score-1e9) == 0 exactly, so they are dropped and the k/v streams padded to
SK = ceil(alive/128)*128 with masked pad rows). q/k/v and all weights are cast
to bf16 on the host so the device can (a) transpose X via the DMA XBAR engine
(2-byte dtype requirement) instead of PE+DVE, and (b) run all projections at
full bf16 PE rate.

Per-core dataflow (everything stays transposed until the output projection):
  X.T via DMA-transpose -> Q.T/K.T = W.T @ X.T (bf16), V natural (bf16, ones
  column appended) -> scores.T = K @ Q.T -> exp+mask+scale in one ACT op ->
  ctxU.T = V'.T @ expS.T (last row = softmax denominator) -> normalize
  (gpsimd broadcast + DVE reciprocal_approx_fast) -> out = ctx.T.T @ Wo + bo.
"""

import sys

if "/opt/trn_rl_repo" not in sys.path:
    sys.path.append("/opt/trn_rl_repo")

import numpy as np
import ml_dtypes

import concourse.bass as bass
import concourse.bacc as bacc
import concourse.tile as tile
from concourse import mybir
from concourse.bass import ts

F32 = mybir.dt.float32
F32R = mybir.dt.float32r
BF16 = mybir.dt.bfloat16
I32 = mybir.dt.int32
EXP = mybir.ActivationFunctionType.Exp

P = 128


def build_nc(S=2048, D=1024, DL=512, HD=64, SKT=9):
    """Per-core Bass program. DL = local output dim; SKT = key token tiles."""
    SK = SKT * P  # compacted+padded key tokens
    KD = D // P  # contraction tiles over D
    MT = DL // P  # local d-col tiles
    HL = DL // HD  # local heads
    HPT = P // HD  # heads per 128-partition tile (2)
    NCH = min(512, S)  # q-projection token chunk
    QS = min(1024, S)  # attention q superchunk (<=2 psum banks)
    QH = min(512, QS)  # one-bank half
    NH = QS // QH
    NQ = S // QS
    OC = min(512, D)  # out-proj col chunk
    kchunks = []
    off = 0
    while off < SK:
        w = min(512, SK - off)
        kchunks.append((off, w))
        off += w
    scale = float(1.0 / (np.sqrt(np.float32(HD)) + 1e-8))

    nc = bacc.Bacc("TRN2", target_bir_lowering=False, debug=False)

    xq = nc.dram_tensor("xq", [S, D], BF16, kind="ExternalInput")
    xk = nc.dram_tensor("xk", [SK, D], BF16, kind="ExternalInput")
    xv = nc.dram_tensor("xv", [SK, D], BF16, kind="ExternalInput")
    msk = nc.dram_tensor("msk", [P, SKT], I32, kind="ExternalInput")
    wq = nc.dram_tensor("wq", [D, DL], BF16, kind="ExternalInput")
    wk = nc.dram_tensor("wk", [D, DL], BF16, kind="ExternalInput")
    wv = nc.dram_tensor("wv", [D, DL], BF16, kind="ExternalInput")
    wo = nc.dram_tensor("wo", [DL, D], BF16, kind="ExternalInput")
    bq = nc.dram_tensor("bq", [P, MT], F32, kind="ExternalInput")
    bk = nc.dram_tensor("bk", [P, MT], F32, kind="ExternalInput")
    bv = nc.dram_tensor("bv", [1, DL], F32, kind="ExternalInput")
    bo = nc.dram_tensor("bo", [1, D], F32, kind="ExternalInput")
    out = nc.dram_tensor("out", [S, D], F32, kind="ExternalOutput")

    with tile.TileContext(nc) as tc, nc.allow_low_precision("bf16 compute by design"):
        with (
            tc.tile_pool(name="pers", bufs=1) as pers,
            tc.tile_pool(name="wpool", bufs=1) as wpool,
            tc.tile_pool(name="exp", bufs=6) as ex_pool,
            tc.tile_pool(name="osb", bufs=2) as osb_pool,
            tc.tile_pool(name="small", bufs=2) as small,
        ):
            # ---- constants ----
            ones0 = pers.tile([1, P], F32, tag="ones0")
            nc.gpsimd.memset(ones0[:], 1.0)
            ones = pers.tile([1, P], F32R, tag="ones")
            nc.vector.tensor_copy(out=ones[:], in_=ones0[:])

            mi = pers.tile([P, SKT], I32, tag="mi")
            nc.sync.dma_start(mi[:], msk[:, :])
            mf = pers.tile([P, SKT], F32, tag="mf")
            nc.vector.tensor_copy(out=mf[:], in_=mi[:])
            mb = pers.tile([P, SKT], F32, tag="mb")
            nc.vector.tensor_scalar_mul(mb[:], mf[:], -1.0e9)

            bqs = pers.tile([P, MT], F32, tag="bqs")
            nc.sync.dma_start(bqs[:], bq[:, :])
            bks = pers.tile([P, MT], F32, tag="bks")
            nc.sync.dma_start(bks[:], bk[:, :])
            bvstg = small.tile([1, D], F32, tag="bstg", name="bvstg")
            nc.sync.dma_start(bvstg[0:1, 0:DL], bv[:, :])
            bvs = pers.tile([1, DL], F32R, tag="bvs")
            nc.vector.tensor_copy(out=bvs[:], in_=bvstg[0:1, 0:DL])
            bostg = small.tile([1, D], F32, tag="bstg", name="bostg")
            nc.sync.dma_start(bostg[:], bo[:, :])
            bos = pers.tile([1, D], F32R, tag="bos")
            nc.vector.tensor_copy(out=bos[:], in_=bostg[:])
            bvb = pers.tile([P, DL], F32, tag="bvb")
            bob = pers.tile([P, D], F32, tag="bob")

            wos = pers.tile([P, MT, D], BF16, tag="wos")
            nc.sync.dma_start(wos[:], wo.rearrange("(m p) n -> p m n", p=P))

            # ---- X.T via DMA XBAR transpose (no PE/DVE involvement) ----
            XKT = pers.tile([P, KD, SK], BF16, tag="xkt")
            for kk in range(KD):
                nc.sync.dma_start_transpose(XKT[:, kk, :], xk[:, ts(kk, P)])
            XVT = pers.tile([P, KD, SK], BF16, tag="xvt")
            for kk in range(KD):
                nc.sync.dma_start_transpose(XVT[:, kk, :], xv[:, ts(kk, P)])
            XQT = pers.tile([P, KD, S], BF16, tag="xqt")
            for kk in range(KD):
                nc.sync.dma_start_transpose(XQT[:, kk, :], xq[:, ts(kk, P)])

            # persistent activation stores
            KT = [pers.tile([P, SK], BF16, tag=f"kt{m}", name=f"kt{m}") for m in range(MT)]
            QT = [pers.tile([P, S], BF16, tag=f"qt{m}", name=f"qt{m}") for m in range(MT)]
            CT = [pers.tile([P, S], BF16, tag=f"ct{m}", name=f"ct{m}") for m in range(MT)]
            VP = [pers.tile([P, HL * (HD + 1)], BF16, tag=f"vp{t}", name=f"vp{t}") for t in range(SKT)]
            for t in range(SKT):
                nc.gpsimd.memset(VP[t][:], 1.0)

            def load_w(wdram):
                w = wpool.tile([P, KD, DL], BF16, tag="w", name="w")
                nc.sync.dma_start(w[:], wdram.rearrange("(k p) n -> p k n", p=P))
                return w

            def qkproj_units(XT, wsb, bias_sb, dst_tiles, chunks, acc_pool):
                """dst[m][:, chunk] = (x @ w + b).T chunk; yields per (chunk, m)."""
                for c0, cw in chunks:
                    for m in range(MT):
                        acc = acc_pool.tile([P, QH], F32, tag="acc", name="pacc")
                        for kk in range(KD):
                            nc.tensor.matmul(
                                acc[:, 0:cw],
                                lhsT=wsb[:, kk, ts(m, P)],
                                rhs=XT[:, kk, c0 : c0 + cw],
                                start=(kk == 0),
                                stop=(kk == KD - 1),
                            )
                        nc.vector.tensor_scalar_add(
                            dst_tiles[m][:, c0 : c0 + cw],
                            acc[:, 0:cw],
                            bias_sb[:, m : m + 1],
                        )
                        yield

            def vproj(wsb, acc_pool):
                """VP[t][:, h*(HD+1):+HD] = (xv @ wv + bv)[t-tile, h-slice]."""
                for t in range(SKT):
                    acc = acc_pool.tile([P, DL], F32, tag="acc", name="vacc")
                    for kk in range(KD):
                        nc.tensor.matmul(
                            acc[:],
                            lhsT=XVT[:, kk, ts(t, P)],
                            rhs=wsb[:, kk, :],
                            start=(kk == 0),
                            stop=(kk == KD - 1),
                        )
                    for h in range(HL):
                        nc.vector.tensor_add(
                            VP[t][:, h * (HD + 1) : h * (HD + 1) + HD],
                            acc[:, ts(h, HD)],
                            bvb[:, ts(h, HD)],
                        )

            def attention(qq, sc_pool, cx_pool, filler=None, pump_every=8):
                it = 0
                pending = []  # deferred normalize work (broadcast/recip/mul)
                for hp in range(HL // HPT):  # head pairs share a KT/QT tile
                    for q5 in range(NH):
                        col0 = qq * QS + q5 * QH
                        cxs = [
                            cx_pool.tile([HD + 1, QH], F32, tag="cx", name="cx")
                            for _ in range(HPT)
                        ]
                        for kt in range(SKT):
                            # one PSUM supertile holds both heads' score chunk;
                            # the two K=64 matmuls use disjoint row groups, one
                            # ACT exp covers both
                            sc = sc_pool.tile([P, HPT * QH], F32, tag="sc")
                            for u in range(HPT):
                                mo = u * HD
                                nc.tensor.matmul(
                                    sc[:, ts(u, QH)],
                                    lhsT=KT[hp][mo : mo + HD, ts(kt, P)],
                                    rhs=QT[hp][mo : mo + HD, col0 : col0 + QH],
                                    start=True,
                                    stop=True,
                                )
                            ex = ex_pool.tile([P, HPT * QH], BF16, tag="ex")
                            nc.scalar.activation(
                                ex[:], sc[:], EXP, bias=mb[:, kt : kt + 1], scale=scale
                            )
                            for u in range(HPT):
                                h = hp * HPT + u
                                nc.tensor.matmul(
                                    cxs[u][:],
                                    lhsT=VP[kt][:, h * (HD + 1) : (h + 1) * (HD + 1)],
                                    rhs=ex[:, ts(u, QH)],
                                    start=(kt == 0),
                                    stop=(kt == SKT - 1),
                                )
                            it += 1
                            if filler is not None and it % pump_every == 0:
                                next(filler, None)
                        prev_tails = pending
                        pending = []
                        for u in range(HPT):
                            mo = u * HD
                            # the cheap DVE copy frees the ctx PSUM slot;
                            # broadcast/recip/mul are deferred one unit so
                            # nothing waits on them
                            stg = small.tile(
                                [HD + 1, QH], F32, tag="stg", name="stg", bufs=4
                            )
                            nc.vector.tensor_copy(out=stg[:], in_=cxs[u][:])

                            def tail(hp=hp, mo=mo, col0=col0, stg=stg):
                                # denominator row lives on partition HD; gpsimd
                                # broadcast and custom-DVE ops only read base
                                # partition 0, so DMA it there first
                                den = small.tile([1, QH], F32, tag="den", bufs=2)
                                nc.sync.dma_start(den[0:1, :], stg[HD : HD + 1, :])
                                rec1 = small.tile([1, QH], F32, tag="rec1", bufs=2)
                                nc.vector.reciprocal_approx_fast(rec1[:], den[:])
                                rec = small.tile([HD, QH], F32, tag="rec", bufs=2)
                                nc.gpsimd.partition_broadcast(rec[:], rec1[0:1, :])
                                if mo == 0:
                                    nc.vector.tensor_mul(
                                        CT[hp][0:HD, col0 : col0 + QH],
                                        stg[0:HD, :],
                                        rec[:],
                                    )
                                else:
                                    tmp = small.tile([HD, QH], BF16, tag="tmp")
                                    nc.vector.tensor_mul(tmp[:], stg[0:HD, :], rec[:])
                                    nc.sync.dma_start(
                                        CT[hp][mo : mo + HD, col0 : col0 + QH], tmp[:]
                                    )

                            pending.append(tail)
                        for fn in prev_tails:
                            fn()

                for fn in pending:
                    fn()

            def outproj_units(qq, acc_pool):
                t0 = qq * (QS // P)
                for t in range(t0, t0 + QS // P):
                    for c in range(D // OC):
                        po = acc_pool.tile([P, OC], F32, tag="acc", name="po")
                        for dd in range(MT):
                            nc.tensor.matmul(
                                po[:],
                                lhsT=CT[dd][:, ts(t, P)],
                                rhs=wos[:, dd, ts(c, OC)],
                                start=(dd == 0),
                                stop=(dd == MT - 1),
                            )
                        osb = osb_pool.tile([P, OC], F32, tag="osb")
                        nc.vector.tensor_add(osb[:], po[:], bob[:, ts(c, OC)])
                        nc.sync.dma_start(out[ts(t, P), ts(c, OC)], osb[:])
                        yield

            def outproj(qq, acc_pool):
                for _ in outproj_units(qq, acc_pool):
                    pass

            # ---- phase 1: K.T and V' (full-SK prerequisites of attention) ----
            with tc.tile_pool(name="ps1", bufs=2, space="PSUM") as ps1:
                for c in range(D // OC):
                    bp = ps1.tile([P, OC], F32, tag="acc", name="bp")
                    nc.tensor.matmul(
                        bp[:], lhsT=ones[0:1, 0:P], rhs=bos[0:1, ts(c, OC)],
                        start=True, stop=True,
                    )
                    nc.vector.tensor_copy(out=bob[:, ts(c, OC)], in_=bp[:])
                bp = ps1.tile([P, DL], F32, tag="acc", name="bp2")
                nc.tensor.matmul(
                    bp[:], lhsT=ones[0:1, 0:P], rhs=bvs[0:1, :], start=True, stop=True
                )
                nc.vector.tensor_copy(out=bvb[:], in_=bp[:])
                wks = load_w(wk)
                for _ in qkproj_units(XKT, wks, bks, KT, kchunks, ps1):
                    pass
                wvs = load_w(wv)
                vproj(wvs, ps1)

            # ---- phase 2: Q.T chunks, attention, out-proj ----
            with (
                tc.tile_pool(name="ps2sc", bufs=2, space="PSUM") as ps2sc,
                tc.tile_pool(name="ps2cx", bufs=2, space="PSUM") as ps2cx,
                tc.tile_pool(name="ps2q", bufs=2, space="PSUM") as ps2q,
            ):
                wqs = load_w(wq)
                CPQ = QS // NCH  # projection chunks per q-superchunk
                qchunks = [(i * NCH, NCH) for i in range(S // NCH)]
                n_its = (HL // HPT) * NH * SKT  # attention kt-iterations per qq

                for _ in qkproj_units(XQT, wqs, bqs, QT, qchunks[:CPQ], ps2q):
                    pass
                for qq in range(NQ):
                    if qq + 1 < NQ:
                        filler = qkproj_units(
                            XQT, wqs, bqs, QT,
                            qchunks[(qq + 1) * CPQ : (qq + 2) * CPQ], ps2q,
                        )
                        n_units = CPQ * MT
                    elif qq >= 1:
                        filler = outproj_units(qq - 1, ps2q)
                        n_units = (QS // P) * (D // OC)
                    else:
                        filler = None
                        n_units = 1
                    attention(
                        qq, ps2sc, ps2cx, filler,
                        pump_every=max(1, n_its // max(n_units, 1)),
                    )
                    if filler is not None:
                        for _ in filler:
                            pass
                outproj(NQ - 1, ps2q)
                for qq in range(NQ - 2):
                    outproj(qq, ps2q)

    nc.compile()
    return nc


_NC_CACHE = {}


def _get_nc(S, D, DL, HD, SKT):
    key = (S, D, DL, HD, SKT)
    if key not in _NC_CACHE:
        _NC_CACHE[key] = build_nc(S, D, DL, HD, SKT)
    return _NC_CACHE[key]


def _shard_inputs(q, k, v, mask, Wq, bq, Wk, bk, Wv, bv, Wo, bo):
    q, k, v = np.asarray(q), np.asarray(k), np.asarray(v)
    mask = np.asarray(mask)
    Wq, Wk, Wv, Wo = np.asarray(Wq), np.asarray(Wk), np.asarray(Wv), np.asarray(Wo)
    bq, bk, bv, bo = np.asarray(bq), np.asarray(bk), np.asarray(bv), np.asarray(bo)

    B, S, D = q.shape  # 4, 2048, 1024
    G = 2  # head-groups (tensor-parallel factor); B*G = 8 cores
    DL = D // G
    MT = DL // P

    bf16 = ml_dtypes.bfloat16
    f32 = np.float32

    # compact keys/values: masked keys contribute exp(score-1e9) == 0 exactly
    m2 = mask[:, 0, 0, :]  # [B, S], 1 = masked
    idxs = [np.nonzero(m2[b] == 0)[0] for b in range(B)]
    SKT = max(1, -(-max(len(ix) for ix in idxs) // P))
    SK = SKT * P

    qb = [np.ascontiguousarray(q[b]).astype(bf16) for b in range(B)]
    kb, vb, mk = [], [], []
    for b in range(B):
        ix = idxs[b]
        n = len(ix)
        kc = np.zeros((SK, D), dtype=bf16)
        kc[:n] = k[b][ix].astype(bf16)
        vc = np.zeros((SK, D), dtype=bf16)
        vc[:n] = v[b][ix].astype(bf16)
        kb.append(kc)
        vb.append(vc)
        mk.append(
            np.ascontiguousarray(
                (np.arange(SK) >= n).astype(np.int32).reshape(SKT, P).T
            )
        )

    in_maps = []
    for c in range(B * G):
        b, g = c // G, c % G
        sl = slice(g * DL, (g + 1) * DL)
        bo_core = bo if g == 0 else np.zeros_like(bo)
        in_maps.append(
            {
                "xq": qb[b],
                "xk": kb[b],
                "xv": vb[b],
                "msk": mk[b],
                "wq": np.ascontiguousarray(Wq[:, sl]).astype(bf16),
                "wk": np.ascontiguousarray(Wk[:, sl]).astype(bf16),
                "wv": np.ascontiguousarray(Wv[:, sl]).astype(bf16),
                "wo": np.ascontiguousarray(Wo[sl, :]).astype(bf16),
                "bq": np.ascontiguousarray(bq[sl].reshape(MT, P).T, dtype=f32),
                "bk": np.ascontiguousarray(bk[sl].reshape(MT, P).T, dtype=f32),
                "bv": np.ascontiguousarray(bv[sl].reshape(1, DL), dtype=f32),
                "bo": np.ascontiguousarray(bo_core.reshape(1, D), dtype=f32),
            }
        )
    return in_maps, SKT


def kernel(q, k, v, mask, Wq, bq, Wk, bk, Wv, bv, Wo, bo):
    from concourse.bass_utils import run_bass_kernel_spmd

    q = np.asarray(q)
    B, S, D = q.shape  # 4, 2048, 1024
    G = 2
    in_maps, SKT = _shard_inputs(q, k, v, mask, Wq, bq, Wk, bk, Wv, bv, Wo, bo)
    nc = _get_nc(S, D, D // G, 64, SKT)

    res = run_bass_kernel_spmd(nc, in_maps, core_ids=list(range(B * G)))
    parts = [r["out"] for r in res.results]
    outf = np.stack([parts[b * G] + parts[b * G + 1] for b in range(B)], axis=0)
    return outf.astype(np.float32)


# revision 9
# speedup vs baseline: 2.2221x; 1.1947x over previous
"""Multi-head attention (B=4, S=2048, D=1024, H=16) on 8 trn2 NeuronCores.

Sharding: 8 cores = 4 batches x 2 head-groups. Core c handles batch c//2 and
heads [8g, 8g+8) where g = c%2 (tensor-parallel: Wq/Wk/Wv column-sliced,
Wo row-sliced). Each core returns a partial output [S, D]; the host sums the
two head-group partials per batch.

Host-side prep: keys/values are COMPACTED per batch (mask==1 keys contribute
exp(score-1e9) == 0 exactly, so they are dropped and the k/v streams padded to
SK = ceil(alive/128)*128 with masked pad rows). q/k/v are transposed to
[D, S]-major on the host and everything is cast to bf16, so the device does
plain sprayed DMA loads and runs all matmuls at full bf16 PE rate.

Per-core dataflow (everything stays transposed until the output projection):
  X.T loaded directly -> Q.T/K.T = W.T @ X.T (bf16), V natural (bf16, ones
  column appended) -> scores.T = K @ Q.T (row-tiled concurrent head pairs) ->
  exp+mask+scale in one ACT op -> ctxU.T = V'.T @ expS.T (last row = softmax
  denominator) -> normalize (DMA denom to partition 0, DVE
  reciprocal_approx_fast, gpsimd broadcast) -> out = ctx.T.T @ Wo + bo.

Schedule: V and m=0 slices of K.T/Q.T are projected up front; the remaining
projection slices, then next-superchunk Q.T, then out-projection chunks are
pumped into the ACT-bound attention loop as PE filler. Attention iterates
q-chunks outermost so out-projection lags attention by one q-chunk.
"""

import sys

if "/opt/trn_rl_repo" not in sys.path:
    sys.path.append("/opt/trn_rl_repo")

import numpy as np
import ml_dtypes

import concourse.bass as bass
import concourse.bacc as bacc
import concourse.tile as tile
from concourse import mybir
from concourse.bass import ts

F32 = mybir.dt.float32
F32R = mybir.dt.float32r
BF16 = mybir.dt.bfloat16
I32 = mybir.dt.int32
EXP = mybir.ActivationFunctionType.Exp

P = 128


def build_nc(S=2048, D=1024, DL=512, HD=64, SKT=9):
    """Per-core Bass program. DL = local output dim; SKT = key token tiles."""
    SK = SKT * P  # compacted+padded key tokens
    KD = D // P  # contraction tiles over D
    MT = DL // P  # local d-col tiles
    HL = DL // HD  # local heads
    HPT = P // HD  # heads per 128-partition tile (2)
    NCH = min(512, S)  # q-projection token chunk
    QS = min(1024, S)  # attention q superchunk
    QH = min(512, QS)  # one-bank column chunk
    NH = QS // QH
    NQ = S // QS
    OC = min(512, D)  # out-proj col chunk
    kchunks = []
    off = 0
    while off < SK:
        w = min(512, SK - off)
        kchunks.append((off, w))
        off += w
    qchunks = [(i * NCH, NCH) for i in range(S // NCH)]
    scale = float(1.0 / (np.sqrt(np.float32(HD)) + 1e-8))

    nc = bacc.Bacc("TRN2", target_bir_lowering=False, debug=False)

    xqt = nc.dram_tensor("xqt", [D, S], BF16, kind="ExternalInput")
    xkt = nc.dram_tensor("xkt", [D, SK], BF16, kind="ExternalInput")
    xvt = nc.dram_tensor("xvt", [D, SK], BF16, kind="ExternalInput")
    msk = nc.dram_tensor("msk", [P, SKT], I32, kind="ExternalInput")
    wq = nc.dram_tensor("wq", [D, DL], BF16, kind="ExternalInput")
    wk = nc.dram_tensor("wk", [D, DL], BF16, kind="ExternalInput")
    wv = nc.dram_tensor("wv", [D, DL], BF16, kind="ExternalInput")
    wo = nc.dram_tensor("wo", [DL, D], BF16, kind="ExternalInput")
    bq = nc.dram_tensor("bq", [P, MT], F32, kind="ExternalInput")
    bk = nc.dram_tensor("bk", [P, MT], F32, kind="ExternalInput")
    bv = nc.dram_tensor("bv", [1, DL], F32, kind="ExternalInput")
    bo = nc.dram_tensor("bo", [1, D], F32, kind="ExternalInput")
    out = nc.dram_tensor("out", [S, D], F32, kind="ExternalOutput")

    with tile.TileContext(nc) as tc, nc.allow_low_precision("bf16 compute by design"):
        with (
            tc.tile_pool(name="pers", bufs=1) as pers,
            tc.tile_pool(name="wpool", bufs=1) as wpool,
            tc.tile_pool(name="exp", bufs=4) as ex_pool,
            tc.tile_pool(name="osb", bufs=2) as osb_pool,
            tc.tile_pool(name="small", bufs=2) as small,
        ):
            # ---- constants ----
            ones0 = pers.tile([1, P], F32, tag="ones0")
            nc.gpsimd.memset(ones0[:], 1.0)
            ones = pers.tile([1, P], F32R, tag="ones")
            nc.vector.tensor_copy(out=ones[:], in_=ones0[:])

            mi = pers.tile([P, SKT], I32, tag="mi")
            nc.sync.dma_start(mi[:], msk[:, :])
            mf = pers.tile([P, SKT], F32, tag="mf")
            nc.vector.tensor_copy(out=mf[:], in_=mi[:])
            mb = pers.tile([P, SKT], F32, tag="mb")
            nc.vector.tensor_scalar_mul(mb[:], mf[:], -1.0e9)

            bqs = pers.tile([P, MT], F32, tag="bqs")
            nc.sync.dma_start(bqs[:], bq[:, :])
            bks = pers.tile([P, MT], F32, tag="bks")
            nc.sync.dma_start(bks[:], bk[:, :])
            bvstg = small.tile([1, DL], F32, tag="bvstg", name="bvstg", bufs=1)
            nc.sync.dma_start(bvstg[0:1, :], bv[:, :])
            bvs = pers.tile([1, DL], F32R, tag="bvs")
            nc.vector.tensor_copy(out=bvs[:], in_=bvstg[0:1, :])
            bostg = small.tile([1, D], F32, tag="bostg", name="bostg", bufs=1)
            nc.sync.dma_start(bostg[:], bo[:, :])
            bos = pers.tile([1, D], F32R, tag="bos")
            nc.vector.tensor_copy(out=bos[:], in_=bostg[:])
            bvb = pers.tile([P, DL], F32, tag="bvb")
            bob = pers.tile([P, D], F32, tag="bob")

            wos = pers.tile([P, MT, D], BF16, tag="wos")
            nc.sync.dma_start(wos[:], wo.rearrange("(m p) n -> p m n", p=P))

            # ---- X.T loads (host pre-transposed, bf16) ----
            XKT = pers.tile([P, KD, SK], BF16, tag="xkt")
            nc.sync.dma_start(XKT[:], xkt.rearrange("(k p) s -> p k s", p=P))
            XVT = pers.tile([P, KD, SK], BF16, tag="xvt")
            nc.sync.dma_start(XVT[:], xvt.rearrange("(k p) s -> p k s", p=P))
            XQT = pers.tile([P, KD, S], BF16, tag="xqt")
            nc.sync.dma_start(XQT[:], xqt.rearrange("(k p) s -> p k s", p=P))

            # persistent activation stores
            KT = [pers.tile([P, SK], BF16, tag=f"kt{m}", name=f"kt{m}") for m in range(MT)]
            QT = [pers.tile([P, S], BF16, tag=f"qt{m}", name=f"qt{m}") for m in range(MT)]
            CT = [pers.tile([P, S], BF16, tag=f"ct{m}", name=f"ct{m}") for m in range(MT)]
            VP = [pers.tile([P, HL * (HD + 1)], BF16, tag=f"vp{t}", name=f"vp{t}") for t in range(SKT)]
            for t in range(SKT):
                nc.gpsimd.memset(VP[t][:], 1.0)

            def load_w(wdram):
                w = wpool.tile([P, KD, DL], BF16, tag="w", name="w")
                nc.sync.dma_start(w[:], wdram.rearrange("(k p) n -> p k n", p=P))
                return w

            def proj_unit(XT, wsb, bias_sb, dst_tiles, c0, cw, m, acc_pool):
                """dst[m][:, c0:c0+cw] = ((x @ w).T + b)[m-rows, chunk]."""
                acc = acc_pool.tile([P, QH], F32, tag="acc", name="pacc")
                for kk in range(KD):
                    nc.tensor.matmul(
                        acc[:, 0:cw],
                        lhsT=wsb[:, kk, ts(m, P)],
                        rhs=XT[:, kk, c0 : c0 + cw],
                        start=(kk == 0),
                        stop=(kk == KD - 1),
                    )
                nc.vector.tensor_scalar_add(
                    dst_tiles[m][:, c0 : c0 + cw], acc[:, 0:cw], bias_sb[:, m : m + 1]
                )

            def vproj(wsb, acc_pool):
                """VP[t][:, h*(HD+1)+1:+HD] = (xv @ wv + bv)[t-tile, h-slice]."""
                for t in range(SKT):
                    acc = acc_pool.tile([P, DL], F32, tag="acc", name="vacc")
                    for kk in range(KD):
                        nc.tensor.matmul(
                            acc[:],
                            lhsT=XVT[:, kk, ts(t, P)],
                            rhs=wsb[:, kk, :],
                            start=(kk == 0),
                            stop=(kk == KD - 1),
                        )
                    for h in range(HL):
                        nc.vector.tensor_add(
                            VP[t][:, h * (HD + 1) : h * (HD + 1) + HD],
                            acc[:, ts(h, HD)],
                            bvb[:, ts(h, HD)],
                        )

            def attn_chunk(qq, q5, sc_pool, cx_pool, pending, filler=None,
                           pump_every=8, pump_offset=0):
                """Attention for q-columns [qq*QS + q5*QH, +QH), all head pairs."""
                # flush tails carried over from the previous chunk so filler
                # out-projections see completed CT columns
                for fn in pending:
                    fn()
                del pending[:]
                it = 0
                col0 = qq * QS + q5 * QH
                for hp in range(HL // HPT):
                    cxs = [
                        cx_pool.tile([HD + 1, QH], F32, tag="cx", name="cx")
                        for _ in range(HPT)
                    ]
                    for kt in range(SKT):
                        sc = sc_pool.tile([P, HPT * QH], F32, tag="sc")
                        for u in range(HPT):
                            mo = u * HD
                            nc.tensor.matmul(
                                sc[:, ts(u, QH)],
                                lhsT=KT[hp][mo : mo + HD, ts(kt, P)],
                                rhs=QT[hp][mo : mo + HD, col0 : col0 + QH],
                                start=True,
                                stop=True,
                            )
                        ex = ex_pool.tile([P, HPT * QH], BF16, tag="ex")
                        nc.scalar.activation(
                            ex[:], sc[:], EXP, bias=mb[:, kt : kt + 1], scale=scale
                        )
                        for u in range(HPT):
                            h = hp * HPT + u
                            nc.tensor.matmul(
                                cxs[u][:],
                                lhsT=VP[kt][:, h * (HD + 1) : (h + 1) * (HD + 1)],
                                rhs=ex[:, ts(u, QH)],
                                start=(kt == 0),
                                stop=(kt == SKT - 1),
                            )
                        it += 1
                        if (
                            filler is not None
                            and it > pump_offset
                            and (it - pump_offset) % pump_every == 0
                        ):
                            next(filler, None)
                    prev_tails = pending[:]
                    del pending[:]
                    for u in range(HPT):
                        mo = u * HD
                        # cheap DVE copy frees the ctx PSUM slot; the rest of
                        # the normalize is deferred one unit
                        stg = small.tile(
                            [HD + 1, QH], F32, tag="stg", name="stg", bufs=4
                        )
                        nc.vector.tensor_copy(out=stg[:], in_=cxs[u][:])

                        def tail(hp=hp, mo=mo, col0=col0, stg=stg):
                            # denominator row lives on partition HD; gpsimd
                            # broadcast and custom-DVE ops only read base
                            # partition 0, so DMA it there first
                            den = small.tile([1, QH], F32, tag="den", bufs=2)
                            nc.sync.dma_start(den[0:1, :], stg[HD : HD + 1, :])
                            rec1 = small.tile([1, QH], F32, tag="rec1", bufs=2)
                            nc.vector.reciprocal_approx_fast(rec1[:], den[:])
                            rec = small.tile([HD, QH], F32, tag="rec", bufs=2)
                            nc.gpsimd.partition_broadcast(rec[:], rec1[0:1, :])
                            if mo == 0:
                                nc.vector.tensor_mul(
                                    CT[hp][0:HD, col0 : col0 + QH], stg[0:HD, :], rec[:]
                                )
                            else:
                                tmp = small.tile([HD, QH], BF16, tag="tmp")
                                nc.vector.tensor_mul(tmp[:], stg[0:HD, :], rec[:])
                                nc.sync.dma_start(
                                    CT[hp][mo : mo + HD, col0 : col0 + QH], tmp[:]
                                )

                        pending.append(tail)
                    for fn in prev_tails:
                        fn()
                # drain remaining filler units
                if filler is not None:
                    for _ in filler:
                        pass

            def outproj_units(qq, q5, acc_pool):
                """Out-projection for the q-token window covered by (qq, q5)."""
                t0 = (qq * QS + q5 * QH) // P
                for t in range(t0, t0 + QH // P):
                    for c in range(D // OC):
                        po = acc_pool.tile([P, OC], F32, tag="acc", name="po")
                        for dd in range(MT):
                            nc.tensor.matmul(
                                po[:],
                                lhsT=CT[dd][:, ts(t, P)],
                                rhs=wos[:, dd, ts(c, OC)],
                                start=(dd == 0),
                                stop=(dd == MT - 1),
                            )
                        osb = osb_pool.tile([P, OC], F32, tag="osb")
                        nc.vector.tensor_add(osb[:], po[:], bob[:, ts(c, OC)])
                        nc.sync.dma_start(out[ts(t, P), ts(c, OC)], osb[:])
                        yield

            # ---- phase 1: V', K.T m=0, Q.T sc0 m=0 (attention prerequisites)
            with tc.tile_pool(name="ps1", bufs=2, space="PSUM") as ps1:
                for c in range(D // OC):
                    bp = ps1.tile([P, OC], F32, tag="acc", name="bp")
                    nc.tensor.matmul(
                        bp[:], lhsT=ones[0:1, 0:P], rhs=bos[0:1, ts(c, OC)],
                        start=True, stop=True,
                    )
                    nc.vector.tensor_copy(out=bob[:, ts(c, OC)], in_=bp[:])
                bp = ps1.tile([P, DL], F32, tag="acc", name="bp2")
                nc.tensor.matmul(
                    bp[:], lhsT=ones[0:1, 0:P], rhs=bvs[0:1, :], start=True, stop=True
                )
                nc.vector.tensor_copy(out=bvb[:], in_=bp[:])
                wks = load_w(wk)
                wvs = wpool.tile([P, KD, DL], BF16, tag="w2", name="w2")
                nc.sync.dma_start(wvs[:], wv.rearrange("(k p) n -> p k n", p=P))
                for c0, cw in kchunks:
                    proj_unit(XKT, wks, bks, KT, c0, cw, 0, ps1)
                vproj(wvs, ps1)

            # ---- phase 2: attention with projection/out-proj filler ----
            with (
                tc.tile_pool(name="ps2sc", bufs=2, space="PSUM") as ps2sc,
                tc.tile_pool(name="ps2cx", bufs=2, space="PSUM") as ps2cx,
                tc.tile_pool(name="ps2q", bufs=2, space="PSUM") as ps2q,
            ):
                wqs = wpool.tile([P, KD, DL], BF16, tag="w3", name="w3")
                nc.sync.dma_start(wqs[:], wq.rearrange("(k p) n -> p k n", p=P))
                CPQ = QS // NCH  # q-proj chunks per superchunk
                n_its = (HL // HPT) * SKT  # attention kt-iterations per chunk
                for c0, cw in qchunks[:CPQ]:
                    proj_unit(XQT, wqs, bqs, QT, c0, cw, 0, ps2q)

                def units(specs):
                    for XT, wsb, bsb, dst, c0, cw, m in specs:
                        proj_unit(XT, wsb, bsb, dst, c0, cw, m, ps2q)
                        yield

                def _mix(a, b):
                    from itertools import chain, zip_longest

                    yield from chain.from_iterable(zip_longest(a, b))

                # chunk sequence: (0,0), (0,1), ..., (NQ-1, NH-1).
                # filler plans keep QT[m]/KT[m] producers strictly ahead of the
                # attention head pair that consumes them (in-order PE queue).
                pending = []
                chunks2 = [(qq, q5) for qq in range(NQ) for q5 in range(NH)]
                for ci, (qq, q5) in enumerate(chunks2):
                    if ci == 0:
                        # remaining K.T slices + this chunk's Q.T column slices
                        specs = []
                        for m in range(1, MT):
                            specs.append((XQT, wqs, bqs, QT, qchunks[0][0], qchunks[0][1], m))
                            for c0, cw in kchunks:
                                specs.append((XKT, wks, bks, KT, c0, cw, m))
                        filler = units(specs)
                        pe, po = 2, 0
                    else:
                        specs = []
                        if ci == 1 and NH * NCH >= QS:
                            # q5=1 column slices for m>=1 (needed by this
                            # chunk's later head pairs)
                            for m in range(1, MT):
                                specs.append(
                                    (XQT, wqs, bqs, QT, qchunks[1][0], qchunks[1][1], m)
                                )
                        if qq + 1 < NQ and q5 == NH - 1:
                            # next superchunk's Q.T, all m
                            for m in range(MT):
                                for c0, cw in qchunks[(qq + 1) * CPQ : (qq + 2) * CPQ]:
                                    specs.append((XQT, wqs, bqs, QT, c0, cw, m))
                        pq, p5 = chunks2[ci - 1]
                        ogen = outproj_units(pq, p5, ps2q)
                        filler = _mix(units(specs), ogen) if specs else ogen
                        nun = len(specs) + QH // P * (D // OC)
                        pe, po = max(1, (n_its - 2) // nun), 2
                    attn_chunk(
                        qq, q5, ps2sc, ps2cx, pending, filler,
                        pump_every=pe, pump_offset=po,
                    )
                for fn in pending:
                    fn()
                for _ in outproj_units(NQ - 1, NH - 1, ps2q):
                    pass

    nc.compile()
    return nc


_NC_CACHE = {}


def _get_nc(S, D, DL, HD, SKT):
    key = (S, D, DL, HD, SKT)
    if key not in _NC_CACHE:
        _NC_CACHE[key] = build_nc(S, D, DL, HD, SKT)
    return _NC_CACHE[key]


def _shard_inputs(q, k, v, mask, Wq, bq, Wk, bk, Wv, bv, Wo, bo):
    q, k, v = np.asarray(q), np.asarray(k), np.asarray(v)
    mask = np.asarray(mask)
    Wq, Wk, Wv, Wo = np.asarray(Wq), np.asarray(Wk), np.asarray(Wv), np.asarray(Wo)
    bq, bk, bv, bo = np.asarray(bq), np.asarray(bk), np.asarray(bv), np.asarray(bo)

    B, S, D = q.shape  # 4, 2048, 1024
    G = 2  # head-groups (tensor-parallel factor); B*G = 8 cores
    DL = D // G
    MT = DL // P

    bf16 = ml_dtypes.bfloat16
    f32 = np.float32

    # compact keys/values: masked keys contribute exp(score-1e9) == 0 exactly
    m2 = mask[:, 0, 0, :]  # [B, S], 1 = masked
    idxs = [np.nonzero(m2[b] == 0)[0] for b in range(B)]
    SKT = max(1, -(-max(len(ix) for ix in idxs) // P))
    SK = SKT * P

    qt = [np.ascontiguousarray(q[b].T.astype(bf16)) for b in range(B)]
    kt, vt, mk = [], [], []
    for b in range(B):
        ix = idxs[b]
        n = len(ix)
        kc = np.zeros((D, SK), dtype=bf16)
        kc[:, :n] = k[b][ix].T.astype(bf16)
        vc = np.zeros((D, SK), dtype=bf16)
        vc[:, :n] = v[b][ix].T.astype(bf16)
        kt.append(kc)
        vt.append(vc)
        mk.append(
            np.ascontiguousarray(
                (np.arange(SK) >= n).astype(np.int32).reshape(SKT, P).T
            )
        )

    in_maps = []
    for c in range(B * G):
        b, g = c // G, c % G
        sl = slice(g * DL, (g + 1) * DL)
        bo_core = bo if g == 0 else np.zeros_like(bo)
        in_maps.append(
            {
                "xqt": qt[b],
                "xkt": kt[b],
                "xvt": vt[b],
                "msk": mk[b],
                "wq": np.ascontiguousarray(Wq[:, sl]).astype(bf16),
                "wk": np.ascontiguousarray(Wk[:, sl]).astype(bf16),
                "wv": np.ascontiguousarray(Wv[:, sl]).astype(bf16),
                "wo": np.ascontiguousarray(Wo[sl, :]).astype(bf16),
                "bq": np.ascontiguousarray(bq[sl].reshape(MT, P).T, dtype=f32),
                "bk": np.ascontiguousarray(bk[sl].reshape(MT, P).T, dtype=f32),
                "bv": np.ascontiguousarray(bv[sl].reshape(1, DL), dtype=f32),
                "bo": np.ascontiguousarray(bo_core.reshape(1, D), dtype=f32),
            }
        )
    return in_maps, SKT


def kernel(q, k, v, mask, Wq, bq, Wk, bk, Wv, bv, Wo, bo):
    from concourse.bass_utils import run_bass_kernel_spmd

    q = np.asarray(q)
    B, S, D = q.shape  # 4, 2048, 1024
    G = 2
    in_maps, SKT = _shard_inputs(q, k, v, mask, Wq, bq, Wk, bk, Wv, bv, Wo, bo)
    nc = _get_nc(S, D, D // G, 64, SKT)

    res = run_bass_kernel_spmd(nc, in_maps, core_ids=list(range(B * G)))
    parts = [r["out"] for r in res.results]
    outf = np.stack([parts[b * G] + parts[b * G + 1] for b in range(B)], axis=0)
    return outf.astype(np.float32)


# revision 12
# speedup vs baseline: 2.3814x; 1.0717x over previous
"""Multi-head attention (B=4, S=2048, D=1024, H=16) on 8 trn2 NeuronCores.

Sharding: 8 cores = 4 batches x 2 head-groups. Core c handles batch c//2 and
heads [8g, 8g+8) where g = c%2 (tensor-parallel: Wq/Wk/Wv column-sliced,
Wo row-sliced). Each core returns a partial output [S, D]; the host sums the
two head-group partials per batch.

Host-side prep: keys/values are COMPACTED per batch (mask==1 keys contribute
exp(score-1e9) == 0 exactly, so they are dropped and the k/v streams padded to
SK = ceil(alive/128)*128 with masked pad rows). q/k/v are transposed to
[D, S]-major on the host and everything is cast to bf16, so the device does
plain sprayed DMA loads and runs all matmuls at full bf16 PE rate.

Per-core dataflow (everything stays transposed until the output projection):
  X.T loaded directly -> Q.T/K.T = W.T @ X.T (bf16), V natural (bf16, ones
  column appended) -> scores.T = K @ Q.T (row-tiled concurrent head pairs) ->
  exp+mask+scale in one ACT op -> ctxU.T = V'.T @ expS.T (last row = softmax
  denominator) -> normalize (DMA denom to partition 0, DVE
  reciprocal_approx_fast, gpsimd broadcast) -> out = ctx.T.T @ Wo + bo.

Schedule: V and m=0 slices of K.T/Q.T are projected up front; the remaining
projection slices, then next-superchunk Q.T, then out-projection chunks are
pumped into the ACT-bound attention loop as PE filler. Attention iterates
q-chunks outermost so out-projection lags attention by one q-chunk.
"""

import sys

if "/opt/trn_rl_repo" not in sys.path:
    sys.path.append("/opt/trn_rl_repo")

import numpy as np
import ml_dtypes

import concourse.bass as bass
import concourse.bacc as bacc
import concourse.tile as tile
from concourse import mybir
from concourse.bass import ts

F32 = mybir.dt.float32
F32R = mybir.dt.float32r
BF16 = mybir.dt.bfloat16
I32 = mybir.dt.int32
EXP = mybir.ActivationFunctionType.Exp

P = 128


def build_nc(S=2048, D=1024, DL=512, HD=64, SKT=9):
    """Per-core Bass program. DL = local output dim; SKT = key token tiles."""
    SK = SKT * P  # compacted+padded key tokens
    KD = D // P  # contraction tiles over D
    MT = DL // P  # local d-col tiles
    HL = DL // HD  # local heads
    HPT = P // HD  # heads per 128-partition tile (2)
    NCH = min(512, S)  # q-projection token chunk
    QS = min(1024, S)  # attention q superchunk
    QH = min(512, QS)  # one-bank column chunk
    NH = QS // QH
    NQ = S // QS
    OC = min(512, D)  # out-proj col chunk
    kchunks = []
    off = 0
    while off < SK:
        w = min(512, SK - off)
        kchunks.append((off, w))
        off += w
    qchunks = [(i * NCH, NCH) for i in range(S // NCH)]
    scale = float(1.0 / (np.sqrt(np.float32(HD)) + 1e-8))

    nc = bacc.Bacc("TRN2", target_bir_lowering=False, debug=False)

    xqt = nc.dram_tensor("xqt", [D, S], BF16, kind="ExternalInput")
    xkt = nc.dram_tensor("xkt", [D, SK], BF16, kind="ExternalInput")
    xvt = nc.dram_tensor("xvt", [D, SK], BF16, kind="ExternalInput")
    msk = nc.dram_tensor("msk", [P, SKT], I32, kind="ExternalInput")
    wq = nc.dram_tensor("wq", [D, DL], BF16, kind="ExternalInput")
    wk = nc.dram_tensor("wk", [D, DL], BF16, kind="ExternalInput")
    wv = nc.dram_tensor("wv", [D, DL], BF16, kind="ExternalInput")
    wo = nc.dram_tensor("wo", [DL, D], BF16, kind="ExternalInput")
    bq = nc.dram_tensor("bq", [P, MT], F32, kind="ExternalInput")
    bk = nc.dram_tensor("bk", [P, MT], F32, kind="ExternalInput")
    bv = nc.dram_tensor("bv", [1, DL], F32, kind="ExternalInput")
    bo = nc.dram_tensor("bo", [1, D], F32, kind="ExternalInput")
    out = nc.dram_tensor("out", [S, D], F32, kind="ExternalOutput")

    with tile.TileContext(nc) as tc, nc.allow_low_precision("bf16 compute by design"):
        with (
            tc.tile_pool(name="pers", bufs=1) as pers,
            tc.tile_pool(name="wpool", bufs=1) as wpool,
            tc.tile_pool(name="exp", bufs=4) as ex_pool,
            tc.tile_pool(name="osb", bufs=2) as osb_pool,
            tc.tile_pool(name="small", bufs=2) as small,
        ):
            # ---- constants ----
            ones0 = pers.tile([1, P], F32, tag="ones0")
            nc.gpsimd.memset(ones0[:], 1.0)
            ones = pers.tile([1, P], F32R, tag="ones")
            nc.vector.tensor_copy(out=ones[:], in_=ones0[:])

            mi = pers.tile([P, SKT], I32, tag="mi")
            nc.sync.dma_start(mi[:], msk[:, :])
            mf = pers.tile([P, SKT], F32, tag="mf")
            nc.vector.tensor_copy(out=mf[:], in_=mi[:])
            mb = pers.tile([P, SKT], F32, tag="mb")
            nc.vector.tensor_scalar_mul(mb[:], mf[:], -1.0e9)

            bqs = pers.tile([P, MT], F32, tag="bqs")
            nc.sync.dma_start(bqs[:], bq[:, :])
            bks = pers.tile([P, MT], F32, tag="bks")
            nc.sync.dma_start(bks[:], bk[:, :])
            bvstg = small.tile([1, DL], F32, tag="bvstg", name="bvstg", bufs=1)
            nc.sync.dma_start(bvstg[0:1, :], bv[:, :])
            bvs = pers.tile([1, DL], F32R, tag="bvs")
            nc.vector.tensor_copy(out=bvs[:], in_=bvstg[0:1, :])
            bostg = small.tile([1, D], F32, tag="bostg", name="bostg", bufs=1)
            nc.sync.dma_start(bostg[:], bo[:, :])
            bos = pers.tile([1, D], F32R, tag="bos")
            nc.vector.tensor_copy(out=bos[:], in_=bostg[:])
            bvb = pers.tile([P, DL], F32, tag="bvb")
            bob = pers.tile([P, D], F32, tag="bob")

            # ---- X.T loads (host pre-transposed, bf16), ordered so the
            # phase-1 gates (XKT+wk, then wv+XVT, then XQT halves) land first;
            # wos (out-proj weights) is only needed mid-attention
            wks = wpool.tile([P, KD, DL], BF16, tag="w", name="w")
            wvs = wpool.tile([P, KD, DL], BF16, tag="w2", name="w2")
            wqs = wpool.tile([P, KD, DL], BF16, tag="w3", name="w3")
            XKT = pers.tile([P, KD, SK], BF16, tag="xkt")
            nc.sync.dma_start(XKT[:], xkt.rearrange("(k p) s -> p k s", p=P))
            nc.sync.dma_start(wks[:], wk.rearrange("(k p) n -> p k n", p=P))
            XVT = pers.tile([P, KD, SK], BF16, tag="xvt")
            nc.sync.dma_start(wvs[:], wv.rearrange("(k p) n -> p k n", p=P))
            nc.sync.dma_start(XVT[:], xvt.rearrange("(k p) s -> p k s", p=P))
            XQT = pers.tile([P, KD, S], BF16, tag="xqt")
            nc.sync.dma_start(XQT[:, :, 0 : S // 2], xqt[:, 0 : S // 2].rearrange("(k p) s -> p k s", p=P))
            nc.sync.dma_start(wqs[:], wq.rearrange("(k p) n -> p k n", p=P))
            nc.sync.dma_start(XQT[:, :, S // 2 : S], xqt[:, S // 2 : S].rearrange("(k p) s -> p k s", p=P))
            wos = pers.tile([P, MT, D], BF16, tag="wos")
            nc.sync.dma_start(wos[:], wo.rearrange("(m p) n -> p m n", p=P))

            # persistent activation stores
            KT = [pers.tile([P, SK], BF16, tag=f"kt{m}", name=f"kt{m}") for m in range(MT)]
            QT = [pers.tile([P, S], BF16, tag=f"qt{m}", name=f"qt{m}") for m in range(MT)]
            CT = [pers.tile([P, S], BF16, tag=f"ct{m}", name=f"ct{m}") for m in range(MT)]
            VP = [pers.tile([P, HL * (HD + 1)], BF16, tag=f"vp{t}", name=f"vp{t}") for t in range(SKT)]
            for t in range(SKT):
                nc.gpsimd.memset(VP[t][:], 1.0)

            def proj_unit(XT, wsb, bias_sb, dst_tiles, c0, cw, m, acc_pool):
                """dst[m][:, c0:c0+cw] = ((x @ w).T + b)[m-rows, chunk]."""
                acc = acc_pool.tile([P, QH], F32, tag="acc", name="pacc")
                for kk in range(KD):
                    nc.tensor.matmul(
                        acc[:, 0:cw],
                        lhsT=wsb[:, kk, ts(m, P)],
                        rhs=XT[:, kk, c0 : c0 + cw],
                        start=(kk == 0),
                        stop=(kk == KD - 1),
                    )
                nc.vector.tensor_scalar_add(
                    dst_tiles[m][:, c0 : c0 + cw], acc[:, 0:cw], bias_sb[:, m : m + 1]
                )

            def vproj(wsb, acc_pool):
                """VP[t][:, h*(HD+1)+1:+HD] = (xv @ wv + bv)[t-tile, h-slice]."""
                for t in range(SKT):
                    acc = acc_pool.tile([P, DL], F32, tag="acc", name="vacc")
                    for kk in range(KD):
                        nc.tensor.matmul(
                            acc[:],
                            lhsT=XVT[:, kk, ts(t, P)],
                            rhs=wsb[:, kk, :],
                            start=(kk == 0),
                            stop=(kk == KD - 1),
                        )
                    for h in range(HL):
                        nc.vector.tensor_add(
                            VP[t][:, h * (HD + 1) : h * (HD + 1) + HD],
                            acc[:, ts(h, HD)],
                            bvb[:, ts(h, HD)],
                        )

            def attn_chunk(qq, q5, sc_pool, cx_pool, pending, filler=None,
                           pump_every=8, pump_offset=0, defer_tails=True):
                """Attention for q-columns [qq*QS + q5*QH, +QH), all head pairs."""
                # flush tails carried over from the previous chunk so filler
                # out-projections see completed CT columns
                for fn in pending:
                    fn()
                del pending[:]
                it = 0
                col0 = qq * QS + q5 * QH
                for hp in range(HL // HPT):
                    cxs = [
                        cx_pool.tile([HD + 1, QH], F32, tag="cx", name="cx")
                        for _ in range(HPT)
                    ]
                    for kt in range(SKT):
                        sc = sc_pool.tile([P, HPT * QH], F32, tag="sc")
                        for u in range(HPT):
                            mo = u * HD
                            nc.tensor.matmul(
                                sc[:, ts(u, QH)],
                                lhsT=KT[hp][mo : mo + HD, ts(kt, P)],
                                rhs=QT[hp][mo : mo + HD, col0 : col0 + QH],
                                start=True,
                                stop=True,
                            )
                        ex = ex_pool.tile([P, HPT * QH], BF16, tag="ex")
                        nc.scalar.activation(
                            ex[:], sc[:], EXP, bias=mb[:, kt : kt + 1], scale=scale
                        )
                        for u in range(HPT):
                            h = hp * HPT + u
                            nc.tensor.matmul(
                                cxs[u][:],
                                lhsT=VP[kt][:, h * (HD + 1) : (h + 1) * (HD + 1)],
                                rhs=ex[:, ts(u, QH)],
                                start=(kt == 0),
                                stop=(kt == SKT - 1),
                            )
                        it += 1
                        if (
                            filler is not None
                            and it > pump_offset
                            and (it - pump_offset) % pump_every == 0
                        ):
                            next(filler, None)
                    prev_tails = pending[:]
                    del pending[:]
                    for u in range(HPT):
                        mo = u * HD
                        # cheap DVE copy frees the ctx PSUM slot; the rest of
                        # the normalize is deferred one unit
                        stg = small.tile(
                            [HD + 1, QH], F32, tag="stg", name="stg", bufs=4
                        )
                        nc.vector.tensor_copy(out=stg[:], in_=cxs[u][:])

                        def tail(hp=hp, mo=mo, col0=col0, stg=stg):
                            # denominator row lives on partition HD; gpsimd
                            # broadcast and custom-DVE ops only read base
                            # partition 0, so DMA it there first
                            den = small.tile([1, QH], F32, tag="den", bufs=2)
                            nc.sync.dma_start(den[0:1, :], stg[HD : HD + 1, :])
                            rec1 = small.tile([1, QH], F32, tag="rec1", bufs=2)
                            nc.vector.reciprocal_approx_fast(rec1[:], den[:])
                            rec = small.tile([HD, QH], F32, tag="rec", bufs=2)
                            nc.gpsimd.partition_broadcast(rec[:], rec1[0:1, :])
                            if mo == 0:
                                nc.vector.tensor_mul(
                                    CT[hp][0:HD, col0 : col0 + QH], stg[0:HD, :], rec[:]
                                )
                            else:
                                tmp = small.tile([HD, QH], BF16, tag="tmp")
                                nc.vector.tensor_mul(tmp[:], stg[0:HD, :], rec[:])
                                nc.sync.dma_start(
                                    CT[hp][mo : mo + HD, col0 : col0 + QH], tmp[:]
                                )

                        if defer_tails:
                            pending.append(tail)
                        else:
                            tail()
                    for fn in prev_tails:
                        fn()
                # drain remaining filler units
                if filler is not None:
                    for _ in filler:
                        pass

            def outproj_units(qq, q5, acc_pool):
                """Out-projection for the q-token window covered by (qq, q5)."""
                t0 = (qq * QS + q5 * QH) // P
                for t in range(t0, t0 + QH // P):
                    for c in range(D // OC):
                        po = acc_pool.tile([P, OC], F32, tag="acc", name="po")
                        for dd in range(MT):
                            nc.tensor.matmul(
                                po[:],
                                lhsT=CT[dd][:, ts(t, P)],
                                rhs=wos[:, dd, ts(c, OC)],
                                start=(dd == 0),
                                stop=(dd == MT - 1),
                            )
                        osb = osb_pool.tile([P, OC], F32, tag="osb")
                        nc.vector.tensor_add(osb[:], po[:], bob[:, ts(c, OC)])
                        nc.sync.dma_start(out[ts(t, P), ts(c, OC)], osb[:])
                        yield

            # ---- phase 1: V', K.T m=0, Q.T sc0 m=0 (attention prerequisites)
            with tc.tile_pool(name="ps1", bufs=2, space="PSUM") as ps1:
                for c in range(D // OC):
                    bp = ps1.tile([P, OC], F32, tag="acc", name="bp")
                    nc.tensor.matmul(
                        bp[:], lhsT=ones[0:1, 0:P], rhs=bos[0:1, ts(c, OC)],
                        start=True, stop=True,
                    )
                    nc.vector.tensor_copy(out=bob[:, ts(c, OC)], in_=bp[:])
                bp = ps1.tile([P, DL], F32, tag="acc", name="bp2")
                nc.tensor.matmul(
                    bp[:], lhsT=ones[0:1, 0:P], rhs=bvs[0:1, :], start=True, stop=True
                )
                nc.vector.tensor_copy(out=bvb[:], in_=bp[:])
                for c0, cw in kchunks:
                    proj_unit(XKT, wks, bks, KT, c0, cw, 0, ps1)
                vproj(wvs, ps1)

            # ---- phase 2: attention with projection/out-proj filler ----
            with (
                tc.tile_pool(name="ps2sc", bufs=2, space="PSUM") as ps2sc,
                tc.tile_pool(name="ps2cx", bufs=2, space="PSUM") as ps2cx,
                tc.tile_pool(name="ps2q", bufs=2, space="PSUM") as ps2q,
            ):
                CPQ = QS // NCH  # q-proj chunks per superchunk
                n_its = (HL // HPT) * SKT  # attention kt-iterations per chunk
                for c0, cw in qchunks[:CPQ]:
                    proj_unit(XQT, wqs, bqs, QT, c0, cw, 0, ps2q)

                def units(specs):
                    for XT, wsb, bsb, dst, c0, cw, m in specs:
                        proj_unit(XT, wsb, bsb, dst, c0, cw, m, ps2q)
                        yield

                def _mix(a, b):
                    from itertools import chain, zip_longest

                    yield from chain.from_iterable(zip_longest(a, b))

                # chunk sequence: (0,0), (0,1), ..., (NQ-1, NH-1).
                # filler plans keep QT[m]/KT[m] producers strictly ahead of the
                # attention head pair that consumes them (in-order PE queue).
                pending = []
                chunks2 = [(qq, q5) for qq in range(NQ) for q5 in range(NH)]
                for ci, (qq, q5) in enumerate(chunks2):
                    if ci == 0:
                        # remaining K.T slices + this chunk's Q.T column slices
                        specs = []
                        for m in range(1, MT):
                            specs.append((XQT, wqs, bqs, QT, qchunks[0][0], qchunks[0][1], m))
                            for c0, cw in kchunks:
                                specs.append((XKT, wks, bks, KT, c0, cw, m))
                        filler = units(specs)
                        pe, po = 2, 0
                    else:
                        specs = []
                        if ci == 1 and NH * NCH >= QS:
                            # q5=1 column slices for m>=1 (needed by this
                            # chunk's later head pairs)
                            for m in range(1, MT):
                                specs.append(
                                    (XQT, wqs, bqs, QT, qchunks[1][0], qchunks[1][1], m)
                                )
                        if qq + 1 < NQ and q5 == NH - 1:
                            # next superchunk's first-needed Q.T columns, all m
                            for m in range(MT):
                                specs.append(
                                    (XQT, wqs, bqs, QT, *qchunks[(qq + 1) * CPQ], m)
                                )
                        if q5 == 0 and qq >= 1:
                            # this superchunk's later Q.T columns were deferred
                            # to here; also pre-project the next superchunk's
                            # later columns if any
                            for m in range(MT):
                                for ck in qchunks[qq * CPQ + 1 : (qq + 1) * CPQ]:
                                    specs.append((XQT, wqs, bqs, QT, *ck, m))
                        pq, p5 = chunks2[ci - 1]
                        ogen = outproj_units(pq, p5, ps2q)
                        filler = _mix(units(specs), ogen) if specs else ogen
                        nun = len(specs) + QH // P * (D // OC)
                        pe, po = max(1, (n_its - 2) // nun), 2
                    attn_chunk(
                        qq, q5, ps2sc, ps2cx, pending, filler,
                        pump_every=pe, pump_offset=po,
                        defer_tails=(ci < len(chunks2) - 1),
                    )
                for fn in pending:
                    fn()
                for _ in outproj_units(NQ - 1, NH - 1, ps2q):
                    pass

    nc.compile()
    return nc


_NC_CACHE = {}


def _get_nc(S, D, DL, HD, SKT):
    key = (S, D, DL, HD, SKT)
    if key not in _NC_CACHE:
        _NC_CACHE[key] = build_nc(S, D, DL, HD, SKT)
    return _NC_CACHE[key]


def _shard_inputs(q, k, v, mask, Wq, bq, Wk, bk, Wv, bv, Wo, bo):
    q, k, v = np.asarray(q), np.asarray(k), np.asarray(v)
    mask = np.asarray(mask)
    Wq, Wk, Wv, Wo = np.asarray(Wq), np.asarray(Wk), np.asarray(Wv), np.asarray(Wo)
    bq, bk, bv, bo = np.asarray(bq), np.asarray(bk), np.asarray(bv), np.asarray(bo)

    B, S, D = q.shape  # 4, 2048, 1024
    G = 2  # head-groups (tensor-parallel factor); B*G = 8 cores
    DL = D // G
    MT = DL // P

    bf16 = ml_dtypes.bfloat16
    f32 = np.float32

    # compact keys/values: masked keys contribute exp(score-1e9) == 0 exactly
    m2 = mask[:, 0, 0, :]  # [B, S], 1 = masked
    idxs = [np.nonzero(m2[b] == 0)[0] for b in range(B)]
    SKT = max(1, -(-max(len(ix) for ix in idxs) // P))
    SK = SKT * P

    qt = [np.ascontiguousarray(q[b].T.astype(bf16)) for b in range(B)]
    kt, vt, mk = [], [], []
    for b in range(B):
        ix = idxs[b]
        n = len(ix)
        kc = np.zeros((D, SK), dtype=bf16)
        kc[:, :n] = k[b][ix].T.astype(bf16)
        vc = np.zeros((D, SK), dtype=bf16)
        vc[:, :n] = v[b][ix].T.astype(bf16)
        kt.append(kc)
        vt.append(vc)
        mk.append(
            np.ascontiguousarray(
                (np.arange(SK) >= n).astype(np.int32).reshape(SKT, P).T
            )
        )

    in_maps = []
    for c in range(B * G):
        b, g = c // G, c % G
        sl = slice(g * DL, (g + 1) * DL)
        bo_core = bo if g == 0 else np.zeros_like(bo)
        in_maps.append(
            {
                "xqt": qt[b],
                "xkt": kt[b],
                "xvt": vt[b],
                "msk": mk[b],
                "wq": np.ascontiguousarray(Wq[:, sl]).astype(bf16),
                "wk": np.ascontiguousarray(Wk[:, sl]).astype(bf16),
                "wv": np.ascontiguousarray(Wv[:, sl]).astype(bf16),
                "wo": np.ascontiguousarray(Wo[sl, :]).astype(bf16),
                "bq": np.ascontiguousarray(bq[sl].reshape(MT, P).T, dtype=f32),
                "bk": np.ascontiguousarray(bk[sl].reshape(MT, P).T, dtype=f32),
                "bv": np.ascontiguousarray(bv[sl].reshape(1, DL), dtype=f32),
                "bo": np.ascontiguousarray(bo_core.reshape(1, D), dtype=f32),
            }
        )
    return in_maps, SKT


def kernel(q, k, v, mask, Wq, bq, Wk, bk, Wv, bv, Wo, bo):
    from concourse.bass_utils import run_bass_kernel_spmd

    q = np.asarray(q)
    B, S, D = q.shape  # 4, 2048, 1024
    G = 2
    in_maps, SKT = _shard_inputs(q, k, v, mask, Wq, bq, Wk, bk, Wv, bv, Wo, bo)
    nc = _get_nc(S, D, D // G, 64, SKT)

    res = run_bass_kernel_spmd(nc, in_maps, core_ids=list(range(B * G)))
    parts = [r["out"] for r in res.results]
    outf = np.stack([parts[b * G] + parts[b * G + 1] for b in range(B)], axis=0)
    return outf.astype(np.float32)
